# revision 18
# baseline (speedup 1.0000x reference)
"""AdaptiveInput (adaptive embedding) kernel for 8 TRN2 NeuronCores.

v3 strategy (trace-driven):
  - Host deals tokens to cores round-robin PER GROUP (stratified): tight
    static caps, balanced cores.  Host does integer bookkeeping only.
  - Groups processed tail-first (tail1 x4, tail2-quad, tail0 x2, head
    LAST) so the 4MB head weight DMA overlaps tail compute.
  - Gathers: SWDGE dma_gather queues 0-2, pads = -1 (skipped transfers).
  - Weights: HWDGE f32 loads on the scalar ring; small converts on ACT,
    big (l0/head) converts on Pool after its desc-gens.
  - h=64 groups (tail1/tail2) transpose TWO 128-token tiles per PE
    transpose: lhsT pair [128, 128] -> tile A rows 0:64, tile B rows
    64:128; weights stacked x2 [128, 1024] so each tile's matmul uses
    its own 64-row quadrant (PE quad-tile, base partitions match).
  - Transposes batched 4-per-PSUM-bank, ONE DVE cast-copy per batch.
  - Matmuls: single N=1024 instruction per contraction slice.
  - Output: contiguous HWDGE writes (sync ring) of [128, ST, 1024] bf16
    stages into partition-major out[128, T_total, 1024]; host unpermutes.
"""
import sys

if "/opt/trn_rl_repo" not in sys.path:
    sys.path.insert(0, "/opt/trn_rl_repo")

import numpy as np

import concourse.bass as bass
import concourse.tile as tile
from concourse import bacc, mybir
from concourse.bass_utils import run_bass_kernel_spmd

# --- problem constants (hardcoded; kernel.py must be self-contained) ---
N_CORES = 8
N_TOK = 4096
D = 1024
CUTOFFS = [0, 10000, 60000, 190000, 250000]
HS = [1024, 256, 64, 16]
SUBRANGE = 32768
ST = 4                          # output tiles per contiguous write chunk
PAD_IDX = -1
MM_N = 512                      # matmul rhs free size (512 = one PSUM bank)

F32 = mybir.dt.float32
BF16 = mybir.dt.bfloat16
I16 = mybir.dt.int16


def _make_groups():
    groups = []
    base = CUTOFFS[2]
    for lo in range(0, CUTOFFS[3] - CUTOFFS[2], SUBRANGE):
        hi = min(lo + SUBRANGE, CUTOFFS[3] - CUTOFFS[2])
        groups.append(dict(cluster=2, lo=base + lo, hi=base + hi, quad=False))
    groups.append(dict(cluster=3, lo=CUTOFFS[3], hi=CUTOFFS[4], quad=True))
    base = CUTOFFS[1]
    for lo in range(0, CUTOFFS[2] - CUTOFFS[1], SUBRANGE):
        hi = min(lo + SUBRANGE, CUTOFFS[2] - CUTOFFS[1])
        groups.append(dict(cluster=1, lo=base + lo, hi=base + hi, quad=False))
    groups.append(dict(cluster=0, lo=0, hi=CUTOFFS[1], quad=False))
    return groups


def _plan(tokens_flat):
    groups = _make_groups()
    per_core = [[] for _ in range(N_CORES)]
    for g in groups:
        idxs = np.nonzero((tokens_flat >= g["lo"]) & (tokens_flat < g["hi"]))[0]
        mx = 0
        for i in range(N_CORES):
            sel = idxs[i::N_CORES]
            per_core[i].append((sel, (tokens_flat[sel] - g["lo"]).astype(np.int64)))
            mx = max(mx, len(sel))
        g["cap"] = max(128, -(-mx // 128) * 128)
        g["C"] = g["cap"] // 128
    t0 = 0
    for g in groups:
        g["tile0"] = t0
        t0 += g["C"]
    return groups, per_core, t0


def _wrap16(vals, cap, pad):
    m = np.full((16, cap // 16), pad, np.int16)
    n = len(vals)
    m[np.arange(n) % 16, np.arange(n) // 16] = vals.astype(np.int16)
    return np.tile(m, (8, 1))


def _build_graph(groups, T_total):
    C2 = next(g["C"] for g in groups if g["quad"])
    U2 = -(-C2 // 2)  # paired-transpose units for the quad group
    S_tot = sum(g["cap"] // 16 for g in groups)

    nc = bacc.Bacc("TRN2", target_bir_lowering=False, debug=False,
                   num_devices=N_CORES, num_swdge_queues=4)

    p_emb = [
        nc.dram_tensor("head_emb", [CUTOFFS[1], 1024], F32, kind="ExternalInput").ap(),
        nc.dram_tensor("tail_emb0", [CUTOFFS[2] - CUTOFFS[1], 256], F32, kind="ExternalInput").ap(),
        nc.dram_tensor("tail_emb1", [CUTOFFS[3] - CUTOFFS[2], 64], F32, kind="ExternalInput").ap(),
        nc.dram_tensor("tail_emb2", [CUTOFFS[4] - CUTOFFS[3], 16], F32, kind="ExternalInput").ap(),
    ]
    p_hwT = nc.dram_tensor("head_wT", [1024, 1024], F32, kind="ExternalInput").ap()
    p_l0 = nc.dram_tensor("tail_lin0", [256, 1024], F32, kind="ExternalInput").ap()
    p_l1 = nc.dram_tensor("tail_lin1", [64, 1024], F32, kind="ExternalInput").ap()
    p_l2 = nc.dram_tensor("tail_lin2", [16, 1024], F32, kind="ExternalInput").ap()
    p_gidx = nc.dram_tensor("gidx", [128, S_tot], I16, kind="ExternalInput").ap()
    p_cnt = nc.dram_tensor("gcnt", [1, 16], mybir.dt.int32, kind="ExternalInput").ap()
    p_mask = nc.dram_tensor("maskT2", [128, U2 * 128], F32, kind="ExternalInput").ap()
    p_ident = nc.dram_tensor("ident", [128, 128], F32, kind="ExternalInput").ap()
    p_out = nc.dram_tensor("out", [128, T_total, 1024], BF16, kind="ExternalOutput").ap()

    with tile.TileContext(nc) as tc:
        from contextlib import ExitStack
        with ExitStack() as ctx:
            cpool = ctx.enter_context(tc.tile_pool(name="const", bufs=1))
            xgpool = ctx.enter_context(tc.tile_pool(name="xg", bufs=1))
            xtpool = ctx.enter_context(tc.tile_pool(name="xt", bufs=3))
            stpool = ctx.enter_context(tc.tile_pool(name="stage", bufs=4))
            ptb_pool = ctx.enter_context(tc.tile_pool(name="ptb", bufs=2, space="PSUM"))
            po_pool = ctx.enter_context(tc.tile_pool(name="pop", bufs=3, space="PSUM"))

            ident = cpool.tile([128, 128], F32, tag="ident")
            gidx_sb = cpool.tile([128, S_tot], I16, tag="gidx")
            cnt_sb = cpool.tile([1, 16], mybir.dt.int32, tag="gcnt")
            mask_sb = cpool.tile([128, U2 * 128], F32, tag="mask")
            nc.sync.dma_start(out=cnt_sb[:], in_=p_cnt[:])
            nc.sync.dma_start(out=gidx_sb[:], in_=p_gidx[:])
            nc.sync.dma_start(out=ident[:], in_=p_ident[:])
            nc.sync.dma_start(out=mask_sb[:], in_=p_mask[:])

            scol = 0
            for g in groups:
                g["scol"] = scol
                scol += g["C"] * 8

            # gathers on queues 0-3, per-core REAL counts from registers
            # (trailing pad slots are never even visited by desc-gen)
            xg_tiles = []
            for gi, g in enumerate(groups):
                if g["quad"]:
                    h_eff = 64
                    in_ap = p_emb[3].rearrange("(q f) h -> q (f h)", f=4)
                else:
                    h_eff = HS[g["cluster"]]
                    cl = g["cluster"]
                    base = CUTOFFS[cl]
                    in_ap = p_emb[cl][g["lo"] - base:g["hi"] - base]
                xg = xgpool.tile([128, g["C"], h_eff], F32, tag=f"xg{gi}")
                nc.gpsimd.dma_gather(
                    out_ap=xg[:], in_ap=in_ap,
                    idxs_ap=gidx_sb[:, g["scol"]:g["scol"] + g["C"] * 8],
                    num_idxs=g["cap"], num_idxs_reg=g["cap"],
                    elem_size=h_eff,
                    queue_num=gi % 4,
                )
                xg_tiles.append((xg, h_eff))

            # ---- weights -------------------------------------------------
            w_l1 = cpool.tile([128, 1024], BF16, tag="w_l1")    # stacked x2
            w_l2 = cpool.tile([128, 1024], BF16, tag="w_l2")    # stacked x8
            w_l0 = cpool.tile([128, 2, 1024], BF16, tag="w_l0")
            w_head = cpool.tile([128, 8, 1024], BF16, tag="w_head")
            w_l1_f = cpool.tile([128, 1024], F32, tag="w_l1_f")
            w_l2_f = cpool.tile([128, 1024], F32, tag="w_l2_f")
            w_l0_f = cpool.tile([128, 2, 1024], F32, tag="w_l0_f")
            w_head_f = cpool.tile([128, 8, 1024], F32, tag="w_head_f")
            hwT_r = p_hwT.rearrange("(k p) d -> p k d", p=128)
            l0_r = p_l0.rearrange("(k p) d -> p k d", p=128)

            nc.scalar.dma_start(out=w_l1_f[0:64, :], in_=p_l1[:])
            nc.scalar.dma_start(out=w_l1_f[64:128, :], in_=p_l1[:])
            for j in range(8):
                nc.sync.dma_start(out=w_l2_f[16 * j:16 * j + 16, :], in_=p_l2[:])
            nc.scalar.dma_start(out=w_l0_f[:], in_=l0_r[:])
            nc.scalar.dma_start(out=w_head_f[:], in_=hwT_r[:])
            nc.scalar.copy(out=w_l1[:], in_=w_l1_f[:])
            nc.scalar.copy(out=w_l2[:], in_=w_l2_f[:])
            nc.gpsimd.tensor_copy(out=w_l0[:], in_=w_l0_f[:])
            nc.gpsimd.tensor_copy(out=w_head[:], in_=w_head_f[:])

            # ---- PE clock warmup: fp32 matmuls during the gather window --
            # (shares the ptb ring slot; recycled by the second real batch)
            warm = ptb_pool.tile([128, 512], F32, tag="ptb")
            for _ in range(4):
                nc.tensor.matmul(out=warm[:], lhsT=ident[:],
                                 rhs=mask_sb[:, :512], start=True, stop=True)

            def rhs_for(g, sub):
                cl = g["cluster"]
                if cl == 0:
                    return lambda k, sl: w_head[:, k, sl]
                if cl == 1:
                    return lambda k, sl: w_l0[:, k, sl]
                if cl == 2:
                    return lambda k, sl: w_l1[64 * sub:64 * sub + 64, sl]
                return lambda k, sl: w_l2[64 * sub:64 * sub + 64, sl]

            # ---- transpose units ----------------------------------------
            # h=64 groups: unit u = tile pair (2u, 2u+1) -> one [128,128]
            #   transpose (tile A rows 0:64, tile B rows 64:128).
            # h>=128 groups: unit = (tile c, k-slice) -> [128,128].
            units = []   # (gi, u_or_c, k, ncols)
            tiles = []   # (gi, c)
            for gi, g in enumerate(groups):
                h_eff = xg_tiles[gi][1]
                K = -(-h_eff // 128)
                g["K"] = K
                g["paired"] = h_eff == 64
                for c in range(g["C"]):
                    tiles.append((gi, c))
                if g["paired"]:
                    for u in range(-(-g["C"] // 2)):
                        nt = min(2, g["C"] - 2 * u)
                        units.append((gi, u, 0, nt * 64))
                else:
                    for c in range(g["C"]):
                        for k in range(K):
                            units.append((gi, c, k, 128))

            batches = []
            cur = []
            for u in units:
                if cur and (len(cur) == 4 or cur[0][0] != u[0]):
                    batches.append(cur)
                    cur = []
                cur.append(u)
            if cur:
                batches.append(cur)

            upos = {}
            for bi, b in enumerate(batches):
                for ui, u in enumerate(b):
                    upos[(u[0], u[1], u[2])] = (bi, ui * 128)

            xt_of_batch = {}

            def emit_batch(bi):
                b = batches[bi]
                gi = b[0][0]
                xg, h_eff = xg_tiles[gi]
                g = groups[gi]
                kk = 128 if g["paired"] else min(128, h_eff)
                used = len(b) * 128
                ptb = ptb_pool.tile([128, 512], F32, tag="ptb")
                for ui, (gi_, uc, k, ncols) in enumerate(b):
                    if g["paired"]:
                        nt = ncols // 64
                        in_ = xg[:, 2 * uc:2 * uc + nt, :]
                    else:
                        cw = min(128, h_eff - 128 * k)
                        in_ = xg[:, uc, 128 * k:128 * k + cw]
                        ncols = cw
                    nc.tensor.transpose(
                        out=ptb[:ncols, 128 * ui:128 * ui + 128],
                        in_=in_,
                        identity=ident[:],
                    )
                xt = xtpool.tile([128, 512], BF16, tag="xt")
                if g["quad"]:
                    u0 = b[0][1]
                    nc.vector.tensor_tensor(
                        out=xt[:kk, :used], in0=ptb[:kk, :used],
                        in1=mask_sb[:, 128 * u0:128 * u0 + used],
                        op=mybir.AluOpType.mult,
                    )
                else:
                    nc.vector.tensor_copy(out=xt[:kk, :used], in_=ptb[:kk, :used])
                xt_of_batch[bi] = xt

            emitted = 0

            def ensure_batch(bi):
                nonlocal emitted
                while emitted <= bi + 1 and emitted < len(batches):
                    emit_batch(emitted)
                    emitted += 1

            copy_alt = 1  # start on ACT: DVE also does the batch casts
            stage = None
            chunk0 = 0
            for ti, (gi, c) in enumerate(tiles):
                g = groups[gi]
                xg, h_eff = xg_tiles[gi]
                K = g["K"]

                po = po_pool.tile([128, 1024], F32, tag="po")
                if g["paired"]:
                    u, sub = c // 2, c % 2
                    bi, col = upos[(gi, u, 0)]
                    ensure_batch(bi)
                    xt = xt_of_batch[bi]
                    rhs = rhs_for(g, sub)
                    lo = 64 * sub
                    for n in range(1024 // MM_N):
                        sl = slice(MM_N * n, MM_N * (n + 1))
                        nc.tensor.matmul(
                            out=po[:, sl], lhsT=xt[lo:lo + 64, col:col + 128],
                            rhs=rhs(0, sl), start=True, stop=True,
                        )
                else:
                    rhs = rhs_for(g, 0)
                    for k in range(K):
                        bi, col = upos[(gi, c, k)]
                        ensure_batch(bi)
                        xt = xt_of_batch[bi]
                        for n in range(1024 // MM_N):
                            sl = slice(MM_N * n, MM_N * (n + 1))
                            nc.tensor.matmul(
                                out=po[:, sl], lhsT=xt[:128, col:col + 128],
                                rhs=rhs(k, sl),
                                start=(k == 0), stop=(k == K - 1),
                            )

                slot = ti - chunk0
                if slot == 0:
                    stage = stpool.tile([128, ST, 1024], BF16, tag="stage")
                if copy_alt % 2 == 0:
                    nc.vector.tensor_copy(out=stage[:, slot, :], in_=po[:])
                else:
                    nc.scalar.copy(out=stage[:, slot, :], in_=po[:])
                copy_alt += 1

                if slot == ST - 1 or ti == len(tiles) - 1:
                    nw = slot + 1
                    nc.sync.dma_start(
                        out=p_out[:, chunk0:chunk0 + nw, :],
                        in_=stage[:, :nw, :],
                    )
                    chunk0 = ti + 1

    nc.compile()
    return nc


_GRAPH_CACHE = {}


def _prepare(tokens_flat):
    groups, per_core, T_total = _plan(tokens_flat)

    key = tuple(g["cap"] for g in groups)
    if key not in _GRAPH_CACHE:
        _GRAPH_CACHE[key] = _build_graph(groups, T_total)
    nc = _GRAPH_CACHE[key]

    C2 = next(g["C"] for g in groups if g["quad"])
    U2 = -(-C2 // 2)
    gidx_np, mask_np, cnt_np = [], [], []
    for i in range(N_CORES):
        gcols = []
        cnts = np.zeros((1, 16), np.int32)
        mask = np.zeros((128, U2 * 128), np.float32)
        for gi, g in enumerate(groups):
            sel, loc = per_core[i][gi]
            cnts[0, gi] = len(sel)
            if g["quad"]:
                gvals = loc // 4
                sub = loc % 4
                for s_i, ssub in enumerate(sub):
                    p, c = s_i % 128, s_i // 128
                    row = 64 * (c % 2) + 16 * ssub
                    mask[row:row + 16, 128 * (c // 2) + p] = 1.0
            else:
                gvals = loc
            gcols.append(_wrap16(gvals, g["cap"], PAD_IDX))
        gidx_np.append(np.concatenate(gcols, axis=1))
        mask_np.append(mask)
        cnt_np.append(cnts)
    return nc, groups, per_core, T_total, gidx_np, mask_np, cnt_np


def run(inputs, trace=False):
    tokens = np.asarray(inputs["tokens"])
    tokens_flat = tokens.reshape(-1).astype(np.int64)
    nc, groups, per_core, T_total, gidx_np, mask_np, cnt_np = _prepare(tokens_flat)

    head_wT = np.ascontiguousarray(np.asarray(inputs["head_w"]).T)
    shared = {
        "head_emb": np.asarray(inputs["head_emb"], np.float32),
        "tail_emb0": np.asarray(inputs["tail_emb0"], np.float32),
        "tail_emb1": np.asarray(inputs["tail_emb1"], np.float32),
        "tail_emb2": np.asarray(inputs["tail_emb2"], np.float32),
        "head_wT": head_wT.astype(np.float32),
        "tail_lin0": np.asarray(inputs["tail_lin0"], np.float32),
        "tail_lin1": np.asarray(inputs["tail_lin1"], np.float32),
        "tail_lin2": np.asarray(inputs["tail_lin2"], np.float32),
        "ident": np.eye(128, dtype=np.float32),
    }
    in_maps = []
    for i in range(N_CORES):
        m = dict(shared)
        m["gidx"] = gidx_np[i]
        m["maskT2"] = mask_np[i]
        m["gcnt"] = cnt_np[i]
        in_maps.append(m)

    res = None
    for attempt in range(3):
        try:
            res = run_bass_kernel_spmd(nc, in_maps, core_ids=list(range(N_CORES)),
                                       trace=trace)
            break
        except Exception:
            if attempt == 2:
                raise
            import time
            time.sleep(2)

    out_flat = np.empty((N_CORES * N_TOK, D), np.float32)
    for i in range(N_CORES):
        r = res.results[i]["out"]  # [128, T_total, 1024] bf16
        for gi, g in enumerate(groups):
            sel, _ = per_core[i][gi]
            n = len(sel)
            if n:
                s = np.arange(n)
                out_flat[sel] = r[s % 128, g["tile0"] + s // 128].astype(np.float32)
    return out_flat.reshape(tokens.shape[0], tokens.shape[1], D), res


def kernel(**inputs):
    out, _ = run(inputs, trace=False)
    return out


# revision 28
# speedup vs baseline: 1.0829x; 1.0829x over previous
"""AdaptiveInput (adaptive embedding) kernel for 8 TRN2 NeuronCores.

v3 strategy (trace-driven):
  - Host deals tokens to cores round-robin PER GROUP (stratified): tight
    static caps, balanced cores.  Host does integer bookkeeping only.
  - Groups processed tail-first (tail1 x4, tail2-quad, tail0 x2, head
    LAST) so the 4MB head weight DMA overlaps tail compute.
  - Gathers: SWDGE dma_gather queues 0-2, pads = -1 (skipped transfers).
  - Weights: HWDGE f32 loads on the scalar ring; small converts on ACT,
    big (l0/head) converts on Pool after its desc-gens.
  - h=64 groups (tail1/tail2) transpose TWO 128-token tiles per PE
    transpose: lhsT pair [128, 128] -> tile A rows 0:64, tile B rows
    64:128; weights stacked x2 [128, 1024] so each tile's matmul uses
    its own 64-row quadrant (PE quad-tile, base partitions match).
  - Transposes batched 4-per-PSUM-bank, ONE DVE cast-copy per batch.
  - Matmuls: single N=1024 instruction per contraction slice.
  - Output: contiguous HWDGE writes (sync ring) of [128, ST, 1024] bf16
    stages into partition-major out[128, T_total, 1024]; host unpermutes.
"""
import sys

if "/opt/trn_rl_repo" not in sys.path:
    sys.path.insert(0, "/opt/trn_rl_repo")

import numpy as np

import concourse.bass as bass
import concourse.tile as tile
from concourse import bacc, mybir
from concourse.bass_utils import run_bass_kernel_spmd

# --- problem constants (hardcoded; kernel.py must be self-contained) ---
N_CORES = 8
N_TOK = 4096
D = 1024
CUTOFFS = [0, 10000, 60000, 190000, 250000]
HS = [1024, 256, 64, 16]
SUBRANGE = 32768
ST = 4                          # output tiles per contiguous write chunk
PAD_IDX = -1
MM_N = 512                      # matmul rhs free size (512 = one PSUM bank)

F32 = mybir.dt.float32
BF16 = mybir.dt.bfloat16
I16 = mybir.dt.int16


def _make_groups():
    groups = []
    base = CUTOFFS[2]
    for lo in range(0, CUTOFFS[3] - CUTOFFS[2], SUBRANGE):
        hi = min(lo + SUBRANGE, CUTOFFS[3] - CUTOFFS[2])
        groups.append(dict(cluster=2, lo=base + lo, hi=base + hi, quad=False))
    groups.append(dict(cluster=3, lo=CUTOFFS[3], hi=CUTOFFS[4], quad=True))
    base = CUTOFFS[1]
    for lo in range(0, CUTOFFS[2] - CUTOFFS[1], SUBRANGE):
        hi = min(lo + SUBRANGE, CUTOFFS[2] - CUTOFFS[1])
        groups.append(dict(cluster=1, lo=base + lo, hi=base + hi, quad=False))
    groups.append(dict(cluster=0, lo=0, hi=CUTOFFS[1], quad=False))
    return groups


def _plan(tokens_flat):
    groups = _make_groups()
    per_core = [[] for _ in range(N_CORES)]
    for g in groups:
        idxs = np.nonzero((tokens_flat >= g["lo"]) & (tokens_flat < g["hi"]))[0]
        mx = 0
        for i in range(N_CORES):
            sel = idxs[i::N_CORES]
            per_core[i].append((sel, (tokens_flat[sel] - g["lo"]).astype(np.int64)))
            mx = max(mx, len(sel))
        g["cap"] = max(128, -(-mx // 128) * 128)
        g["C"] = g["cap"] // 128
    t0 = 0
    for g in groups:
        g["tile0"] = t0
        t0 += g["C"]
    return groups, per_core, t0


def _wrap16(vals, cap, pad):
    m = np.full((16, cap // 16), pad, np.int16)
    n = len(vals)
    m[np.arange(n) % 16, np.arange(n) // 16] = vals.astype(np.int16)
    return np.tile(m, (8, 1))


def _build_graph(groups, T_total):
    C2 = next(g["C"] for g in groups if g["quad"])
    U2 = -(-C2 // 2)  # paired-transpose units for the quad group
    S_tot = sum(g["cap"] // 16 for g in groups)

    nc = bacc.Bacc("TRN2", target_bir_lowering=False, debug=False,
                   num_devices=N_CORES, num_swdge_queues=4)

    p_emb = [
        nc.dram_tensor("head_emb", [CUTOFFS[1], 1024], F32, kind="ExternalInput").ap(),
        nc.dram_tensor("tail_emb0", [CUTOFFS[2] - CUTOFFS[1], 256], F32, kind="ExternalInput").ap(),
        nc.dram_tensor("tail_emb1", [CUTOFFS[3] - CUTOFFS[2], 64], F32, kind="ExternalInput").ap(),
        nc.dram_tensor("tail_emb2", [CUTOFFS[4] - CUTOFFS[3], 16], F32, kind="ExternalInput").ap(),
    ]
    p_hwT = nc.dram_tensor("head_wT", [1024, 1024], F32, kind="ExternalInput").ap()
    p_l0 = nc.dram_tensor("tail_lin0", [256, 1024], F32, kind="ExternalInput").ap()
    p_l1 = nc.dram_tensor("tail_lin1", [64, 1024], F32, kind="ExternalInput").ap()
    p_l2 = nc.dram_tensor("tail_lin2", [16, 1024], F32, kind="ExternalInput").ap()
    p_gidx = nc.dram_tensor("gidx", [128, S_tot], I16, kind="ExternalInput").ap()
    p_mask = nc.dram_tensor("maskT2", [128, U2 * 128], F32, kind="ExternalInput").ap()
    p_ident = nc.dram_tensor("ident", [128, 128], F32, kind="ExternalInput").ap()
    p_out = nc.dram_tensor("out", [128, T_total, 1024], BF16, kind="ExternalOutput").ap()

    with tile.TileContext(nc) as tc:
        from contextlib import ExitStack
        with ExitStack() as ctx:
            cpool = ctx.enter_context(tc.tile_pool(name="const", bufs=1))
            xgpool = ctx.enter_context(tc.tile_pool(name="xg", bufs=1))
            xtpool = ctx.enter_context(tc.tile_pool(name="xt", bufs=3))
            stpool = ctx.enter_context(tc.tile_pool(name="stage", bufs=4))
            ptb_pool = ctx.enter_context(tc.tile_pool(name="ptb", bufs=2, space="PSUM"))
            po_pool = ctx.enter_context(tc.tile_pool(name="pop", bufs=3, space="PSUM"))

            ident = cpool.tile([128, 128], F32, tag="ident")
            gidx_sb = cpool.tile([128, S_tot], I16, tag="gidx")
            mask_sb = cpool.tile([128, U2 * 128], F32, tag="mask")
            nc.sync.dma_start(out=gidx_sb[:], in_=p_gidx[:])
            nc.sync.dma_start(out=ident[:], in_=p_ident[:])
            nc.sync.dma_start(out=mask_sb[:], in_=p_mask[:])

            scol = 0
            for g in groups:
                g["scol"] = scol
                scol += g["C"] * 8

            # gathers on queues 0-3, per-core REAL counts from registers
            # (trailing pad slots are never even visited by desc-gen)
            xg_tiles = []
            for gi, g in enumerate(groups):
                if g["quad"]:
                    h_eff = 64
                    in_ap = p_emb[3].rearrange("(q f) h -> q (f h)", f=4)
                else:
                    h_eff = HS[g["cluster"]]
                    cl = g["cluster"]
                    base = CUTOFFS[cl]
                    in_ap = p_emb[cl][g["lo"] - base:g["hi"] - base]
                xg = xgpool.tile([128, g["C"], h_eff], F32, tag=f"xg{gi}")
                nc.gpsimd.dma_gather(
                    out_ap=xg[:], in_ap=in_ap,
                    idxs_ap=gidx_sb[:, g["scol"]:g["scol"] + g["C"] * 8],
                    num_idxs=g["cap"], num_idxs_reg=g["cap"],
                    elem_size=h_eff,
                    queue_num=gi % 4,
                )
                xg_tiles.append((xg, h_eff))

            # ---- weights -------------------------------------------------
            w_l1 = cpool.tile([128, 1024], BF16, tag="w_l1")    # stacked x2
            w_l2 = cpool.tile([128, 1024], BF16, tag="w_l2")    # stacked x8
            w_l0 = cpool.tile([128, 2, 1024], BF16, tag="w_l0")
            w_head = cpool.tile([128, 8, 1024], BF16, tag="w_head")
            w_l1_f = cpool.tile([128, 1024], F32, tag="w_l1_f")
            w_l2_f = cpool.tile([128, 1024], F32, tag="w_l2_f")
            w_l0_f = cpool.tile([128, 2, 1024], F32, tag="w_l0_f")
            w_head_f = cpool.tile([128, 8, 1024], F32, tag="w_head_f")
            hwT_r = p_hwT.rearrange("(k p) d -> p k d", p=128)
            l0_r = p_l0.rearrange("(k p) d -> p k d", p=128)

            nc.scalar.dma_start(out=w_l1_f[0:64, :], in_=p_l1[:])
            nc.scalar.dma_start(out=w_l1_f[64:128, :], in_=p_l1[:])
            for j in range(8):
                nc.sync.dma_start(out=w_l2_f[16 * j:16 * j + 16, :], in_=p_l2[:])
            nc.scalar.dma_start(out=w_l0_f[:], in_=l0_r[:])
            nc.scalar.dma_start(out=w_head_f[:], in_=hwT_r[:])
            nc.scalar.copy(out=w_l1[:], in_=w_l1_f[:])
            nc.scalar.copy(out=w_l2[:], in_=w_l2_f[:])
            nc.gpsimd.tensor_copy(out=w_l0[:], in_=w_l0_f[:])
            nc.gpsimd.tensor_copy(out=w_head[:], in_=w_head_f[:])

            # ---- PE clock warmup: fp32 matmuls during the gather window --
            # (shares the ptb ring slot; recycled by the second real batch)
            warm = ptb_pool.tile([128, 512], F32, tag="ptb")
            for _ in range(4):
                nc.tensor.matmul(out=warm[:], lhsT=ident[:],
                                 rhs=mask_sb[:, :512], start=True, stop=True)

            def rhs_for(g, sub):
                cl = g["cluster"]
                if cl == 0:
                    return lambda k, sl: w_head[:, k, sl]
                if cl == 1:
                    return lambda k, sl: w_l0[:, k, sl]
                if cl == 2:
                    return lambda k, sl: w_l1[64 * sub:64 * sub + 64, sl]
                return lambda k, sl: w_l2[64 * sub:64 * sub + 64, sl]

            # ---- transpose units ----------------------------------------
            # h=64 groups: unit u = tile pair (2u, 2u+1) -> one [128,128]
            #   transpose (tile A rows 0:64, tile B rows 64:128).
            # h>=128 groups: unit = (tile c, k-slice) -> [128,128].
            units = []   # (gi, u_or_c, k, ncols)
            tiles = []   # (gi, c)
            for gi, g in enumerate(groups):
                h_eff = xg_tiles[gi][1]
                K = -(-h_eff // 128)
                g["K"] = K
                g["paired"] = h_eff == 64
                for c in range(g["C"]):
                    tiles.append((gi, c))
                if g["paired"]:
                    for u in range(-(-g["C"] // 2)):
                        nt = min(2, g["C"] - 2 * u)
                        units.append((gi, u, 0, nt * 64))
                else:
                    for c in range(g["C"]):
                        for k in range(K):
                            units.append((gi, c, k, 128))

            batches = []
            cur = []
            for u in units:
                if cur and (len(cur) == 4 or cur[0][0] != u[0]):
                    batches.append(cur)
                    cur = []
                cur.append(u)
            if cur:
                batches.append(cur)

            upos = {}
            bt0 = {}  # batch -> first consumer tile index (global)
            gt0 = {}
            t_acc = 0
            for gi, g in enumerate(groups):
                gt0[gi] = t_acc
                t_acc += g["C"]
            for bi, b in enumerate(batches):
                for ui, u in enumerate(b):
                    upos[(u[0], u[1], u[2])] = (bi, ui * 128)
                gi, uc = b[0][0], b[0][1]
                first_c = 2 * uc if groups[gi]["paired"] else uc
                bt0[bi] = gt0[gi] + first_c

            xt_of_batch = {}

            # logical-time floors steer the sim-driven Tile scheduler: they
            # stop it hoisting later groups' transposes (and their gather
            # waits) ahead of earlier groups' matmuls, which on HW stalled
            # the whole PE stream on the slowest gather.
            TICK = 0.002  # ms of model time per output tile

            def emit_batch(bi):
                b = batches[bi]
                gi = b[0][0]
                xg, h_eff = xg_tiles[gi]
                g = groups[gi]
                kk = 128 if g["paired"] else min(128, h_eff)
                used = len(b) * 128
                with tc.tile_wait_until(bt0[bi] * TICK):
                    ptb = ptb_pool.tile([128, 512], F32, tag="ptb")
                    for ui, (gi_, uc, k, ncols) in enumerate(b):
                        if g["paired"]:
                            nt = ncols // 64
                            in_ = xg[:, 2 * uc:2 * uc + nt, :]
                        else:
                            cw = min(128, h_eff - 128 * k)
                            in_ = xg[:, uc, 128 * k:128 * k + cw]
                            ncols = cw
                        nc.tensor.transpose(
                            out=ptb[:ncols, 128 * ui:128 * ui + 128],
                            in_=in_,
                            identity=ident[:],
                        )
                    xt = xtpool.tile([128, 512], BF16, tag="xt")
                    if g["quad"]:
                        u0 = b[0][1]
                        nc.vector.tensor_tensor(
                            out=xt[:kk, :used], in0=ptb[:kk, :used],
                            in1=mask_sb[:, 128 * u0:128 * u0 + used],
                            op=mybir.AluOpType.mult,
                        )
                    else:
                        nc.vector.tensor_copy(out=xt[:kk, :used],
                                              in_=ptb[:kk, :used])
                    xt_of_batch[bi] = xt

            emitted = 0

            def ensure_batch(bi):
                nonlocal emitted
                while emitted <= bi + 1 and emitted < len(batches):
                    emit_batch(emitted)
                    emitted += 1

            copy_alt = 1  # start on ACT: DVE also does the batch casts
            stage = None
            chunk0 = 0
            for ti, (gi, c) in enumerate(tiles):
                g = groups[gi]
                xg, h_eff = xg_tiles[gi]
                K = g["K"]

                if g["paired"]:
                    ensure_batch(upos[(gi, c // 2, 0)][0])
                else:
                    ensure_batch(upos[(gi, c, K - 1)][0])
                with tc.tile_wait_until((ti + 2) * TICK):
                    po = po_pool.tile([128, 1024], F32, tag="po")
                    if g["paired"]:
                        u, sub = c // 2, c % 2
                        bi, col = upos[(gi, u, 0)]
                        xt = xt_of_batch[bi]
                        rhs = rhs_for(g, sub)
                        lo = 64 * sub
                        for n in range(1024 // MM_N):
                            sl = slice(MM_N * n, MM_N * (n + 1))
                            nc.tensor.matmul(
                                out=po[:, sl],
                                lhsT=xt[lo:lo + 64, col:col + 128],
                                rhs=rhs(0, sl), start=True, stop=True,
                            )
                    else:
                        rhs = rhs_for(g, 0)
                        for k in range(K):
                            bi, col = upos[(gi, c, k)]
                            xt = xt_of_batch[bi]
                            for n in range(1024 // MM_N):
                                sl = slice(MM_N * n, MM_N * (n + 1))
                                nc.tensor.matmul(
                                    out=po[:, sl], lhsT=xt[:128, col:col + 128],
                                    rhs=rhs(k, sl),
                                    start=(k == 0), stop=(k == K - 1),
                                )

                    slot = ti - chunk0
                    if slot == 0:
                        stage = stpool.tile([128, ST, 1024], BF16, tag="stage")
                    if copy_alt % 2 == 0:
                        nc.vector.tensor_copy(out=stage[:, slot, :], in_=po[:])
                    else:
                        nc.scalar.copy(out=stage[:, slot, :], in_=po[:])
                    copy_alt += 1

                    if slot == ST - 1 or ti == len(tiles) - 1:
                        nw = slot + 1
                        nc.sync.dma_start(
                            out=p_out[:, chunk0:chunk0 + nw, :],
                            in_=stage[:, :nw, :],
                        )
                        chunk0 = ti + 1

    nc.compile()
    return nc


_GRAPH_CACHE = {}


def _prepare(tokens_flat):
    groups, per_core, T_total = _plan(tokens_flat)

    key = tuple(g["cap"] for g in groups)
    if key not in _GRAPH_CACHE:
        _GRAPH_CACHE[key] = _build_graph(groups, T_total)
    nc = _GRAPH_CACHE[key]

    C2 = next(g["C"] for g in groups if g["quad"])
    U2 = -(-C2 // 2)
    gidx_np, mask_np = [], []
    for i in range(N_CORES):
        gcols = []
        mask = np.zeros((128, U2 * 128), np.float32)
        for gi, g in enumerate(groups):
            sel, loc = per_core[i][gi]
            if g["quad"]:
                gvals = loc // 4
                sub = loc % 4
                for s_i, ssub in enumerate(sub):
                    p, c = s_i % 128, s_i // 128
                    row = 64 * (c % 2) + 16 * ssub
                    mask[row:row + 16, 128 * (c // 2) + p] = 1.0
            else:
                gvals = loc
            gcols.append(_wrap16(gvals, g["cap"], PAD_IDX))
        gidx_np.append(np.concatenate(gcols, axis=1))
        mask_np.append(mask)
    return nc, groups, per_core, T_total, gidx_np, mask_np


def run(inputs, trace=False):
    tokens = np.asarray(inputs["tokens"])
    tokens_flat = tokens.reshape(-1).astype(np.int64)
    nc, groups, per_core, T_total, gidx_np, mask_np = _prepare(tokens_flat)

    head_wT = np.ascontiguousarray(np.asarray(inputs["head_w"]).T)
    shared = {
        "head_emb": np.asarray(inputs["head_emb"], np.float32),
        "tail_emb0": np.asarray(inputs["tail_emb0"], np.float32),
        "tail_emb1": np.asarray(inputs["tail_emb1"], np.float32),
        "tail_emb2": np.asarray(inputs["tail_emb2"], np.float32),
        "head_wT": head_wT.astype(np.float32),
        "tail_lin0": np.asarray(inputs["tail_lin0"], np.float32),
        "tail_lin1": np.asarray(inputs["tail_lin1"], np.float32),
        "tail_lin2": np.asarray(inputs["tail_lin2"], np.float32),
        "ident": np.eye(128, dtype=np.float32),
    }
    in_maps = []
    for i in range(N_CORES):
        m = dict(shared)
        m["gidx"] = gidx_np[i]
        m["maskT2"] = mask_np[i]
        in_maps.append(m)

    res = None
    for attempt in range(3):
        try:
            res = run_bass_kernel_spmd(nc, in_maps, core_ids=list(range(N_CORES)),
                                       trace=trace)
            break
        except Exception:
            if attempt == 2:
                raise
            import time
            time.sleep(2)

    out_flat = np.empty((N_CORES * N_TOK, D), np.float32)
    for i in range(N_CORES):
        r = res.results[i]["out"]  # [128, T_total, 1024] bf16
        for gi, g in enumerate(groups):
            sel, _ = per_core[i][gi]
            n = len(sel)
            if n:
                s = np.arange(n)
                out_flat[sel] = r[s % 128, g["tile0"] + s // 128].astype(np.float32)
    return out_flat.reshape(tokens.shape[0], tokens.shape[1], D), res


def kernel(**inputs):
    out, _ = run(inputs, trace=False)
    return out


# revision 30
# speedup vs baseline: 1.1105x; 1.0255x over previous
"""AdaptiveInput (adaptive embedding) kernel for 8 TRN2 NeuronCores.

v3 strategy (trace-driven):
  - Host deals tokens to cores round-robin PER GROUP (stratified): tight
    static caps, balanced cores.  Host does integer bookkeeping only.
  - Groups processed tail-first (tail1 x4, tail2-quad, tail0 x2, head
    LAST) so the 4MB head weight DMA overlaps tail compute.
  - Gathers: SWDGE dma_gather queues 0-2, pads = -1 (skipped transfers).
  - Weights: HWDGE f32 loads on the scalar ring; small converts on ACT,
    big (l0/head) converts on Pool after its desc-gens.
  - h=64 groups (tail1/tail2) transpose TWO 128-token tiles per PE
    transpose: lhsT pair [128, 128] -> tile A rows 0:64, tile B rows
    64:128; weights stacked x2 [128, 1024] so each tile's matmul uses
    its own 64-row quadrant (PE quad-tile, base partitions match).
  - Transposes batched 4-per-PSUM-bank, ONE DVE cast-copy per batch.
  - Matmuls: single N=1024 instruction per contraction slice.
  - Output: contiguous HWDGE writes (sync ring) of [128, ST, 1024] bf16
    stages into partition-major out[128, T_total, 1024]; host unpermutes.
"""
import sys

if "/opt/trn_rl_repo" not in sys.path:
    sys.path.insert(0, "/opt/trn_rl_repo")

import numpy as np

import concourse.bass as bass
import concourse.tile as tile
from concourse import bacc, mybir
from concourse.bass_utils import run_bass_kernel_spmd

# --- problem constants (hardcoded; kernel.py must be self-contained) ---
N_CORES = 8
N_TOK = 4096
D = 1024
CUTOFFS = [0, 10000, 60000, 190000, 250000]
HS = [1024, 256, 64, 16]
SUBRANGE = 32768
ST = 4                          # output tiles per contiguous write chunk
PAD_IDX = -1
MM_N = 512                      # matmul rhs free size (512 = one PSUM bank)

F32 = mybir.dt.float32
BF16 = mybir.dt.bfloat16
I16 = mybir.dt.int16


def _make_groups():
    groups = []
    base = CUTOFFS[2]
    for lo in range(0, CUTOFFS[3] - CUTOFFS[2], SUBRANGE):
        hi = min(lo + SUBRANGE, CUTOFFS[3] - CUTOFFS[2])
        groups.append(dict(cluster=2, lo=base + lo, hi=base + hi, quad=False))
    groups.append(dict(cluster=3, lo=CUTOFFS[3], hi=CUTOFFS[4], quad=True))
    base = CUTOFFS[1]
    for lo in range(0, CUTOFFS[2] - CUTOFFS[1], SUBRANGE):
        hi = min(lo + SUBRANGE, CUTOFFS[2] - CUTOFFS[1])
        groups.append(dict(cluster=1, lo=base + lo, hi=base + hi, quad=False))
    groups.append(dict(cluster=0, lo=0, hi=CUTOFFS[1], quad=False))
    return groups


def _plan(tokens_flat):
    groups = _make_groups()
    per_core = [[] for _ in range(N_CORES)]
    for g in groups:
        idxs = np.nonzero((tokens_flat >= g["lo"]) & (tokens_flat < g["hi"]))[0]
        mx = 0
        for i in range(N_CORES):
            sel = idxs[i::N_CORES]
            per_core[i].append((sel, (tokens_flat[sel] - g["lo"]).astype(np.int64)))
            mx = max(mx, len(sel))
        g["cap"] = max(128, -(-mx // 128) * 128)
        g["C"] = g["cap"] // 128
    t0 = 0
    for g in groups:
        g["tile0"] = t0
        t0 += g["C"]
    return groups, per_core, t0


def _wrap16(vals, cap, pad):
    m = np.full((16, cap // 16), pad, np.int16)
    n = len(vals)
    m[np.arange(n) % 16, np.arange(n) // 16] = vals.astype(np.int16)
    return np.tile(m, (8, 1))


def _build_graph(groups, T_total):
    C2 = next(g["C"] for g in groups if g["quad"])
    U2 = -(-C2 // 2)  # paired-transpose units for the quad group
    S_tot = sum(g["cap"] // 16 for g in groups)

    nc = bacc.Bacc("TRN2", target_bir_lowering=False, debug=False,
                   num_devices=N_CORES, num_swdge_queues=4)

    p_emb = [
        nc.dram_tensor("head_emb", [CUTOFFS[1], 1024], F32, kind="ExternalInput").ap(),
        nc.dram_tensor("tail_emb0", [CUTOFFS[2] - CUTOFFS[1], 256], F32, kind="ExternalInput").ap(),
        nc.dram_tensor("tail_emb1", [CUTOFFS[3] - CUTOFFS[2], 64], F32, kind="ExternalInput").ap(),
        nc.dram_tensor("tail_emb2", [CUTOFFS[4] - CUTOFFS[3], 16], F32, kind="ExternalInput").ap(),
    ]
    p_hwT = nc.dram_tensor("head_wT", [1024, 1024], F32, kind="ExternalInput").ap()
    p_l0 = nc.dram_tensor("tail_lin0", [256, 1024], F32, kind="ExternalInput").ap()
    p_l1 = nc.dram_tensor("tail_lin1", [64, 1024], F32, kind="ExternalInput").ap()
    p_l2 = nc.dram_tensor("tail_lin2", [16, 1024], F32, kind="ExternalInput").ap()
    p_gidx = nc.dram_tensor("gidx", [128, S_tot], I16, kind="ExternalInput").ap()
    p_mask = nc.dram_tensor("maskT2", [128, U2 * 128], F32, kind="ExternalInput").ap()
    p_ident = nc.dram_tensor("ident", [128, 128], F32, kind="ExternalInput").ap()
    p_out = nc.dram_tensor("out", [128, T_total, 1024], BF16, kind="ExternalOutput").ap()

    with tile.TileContext(nc) as tc:
        from contextlib import ExitStack
        with ExitStack() as ctx:
            cpool = ctx.enter_context(tc.tile_pool(name="const", bufs=1))
            xgpool = ctx.enter_context(tc.tile_pool(name="xg", bufs=1))
            xtpool = ctx.enter_context(tc.tile_pool(name="xt", bufs=3))
            stpool = ctx.enter_context(tc.tile_pool(name="stage", bufs=4))
            ptb_pool = ctx.enter_context(tc.tile_pool(name="ptb", bufs=2, space="PSUM"))
            po_pool = ctx.enter_context(tc.tile_pool(name="pop", bufs=3, space="PSUM"))

            ident = cpool.tile([128, 128], F32, tag="ident")
            gidx_sb = cpool.tile([128, S_tot], I16, tag="gidx")
            mask_sb = cpool.tile([128, U2 * 128], F32, tag="mask")

            scol = 0
            for g in groups:
                g["scol"] = scol
                scol += g["C"] * 8

            # per-group gidx loads: each gather's desc-gen waits only on its
            # own tiny slice (a monolithic load straggled behind the 4MB
            # weight stream on the shared SDMA engines)
            for g in groups:
                nc.sync.dma_start(
                    out=gidx_sb[:, g["scol"]:g["scol"] + g["C"] * 8],
                    in_=p_gidx[:, g["scol"]:g["scol"] + g["C"] * 8],
                )
            nc.sync.dma_start(out=ident[:], in_=p_ident[:])
            nc.sync.dma_start(out=mask_sb[:], in_=p_mask[:])

            # gathers on queues 0-3, per-core REAL counts from registers
            # (trailing pad slots are never even visited by desc-gen)
            xg_tiles = []
            for gi, g in enumerate(groups):
                if g["quad"]:
                    h_eff = 64
                    in_ap = p_emb[3].rearrange("(q f) h -> q (f h)", f=4)
                else:
                    h_eff = HS[g["cluster"]]
                    cl = g["cluster"]
                    base = CUTOFFS[cl]
                    in_ap = p_emb[cl][g["lo"] - base:g["hi"] - base]
                xg = xgpool.tile([128, g["C"], h_eff], F32, tag=f"xg{gi}")
                nc.gpsimd.dma_gather(
                    out_ap=xg[:], in_ap=in_ap,
                    idxs_ap=gidx_sb[:, g["scol"]:g["scol"] + g["C"] * 8],
                    num_idxs=g["cap"], num_idxs_reg=g["cap"],
                    elem_size=h_eff,
                    queue_num=gi % 4,
                )
                xg_tiles.append((xg, h_eff))

            # ---- weights -------------------------------------------------
            w_l1 = cpool.tile([128, 1024], BF16, tag="w_l1")    # stacked x2
            w_l2 = cpool.tile([128, 1024], BF16, tag="w_l2")    # stacked x8
            w_l0 = cpool.tile([128, 2, 1024], BF16, tag="w_l0")
            w_head = cpool.tile([128, 8, 1024], BF16, tag="w_head")
            w_l1_f = cpool.tile([128, 1024], F32, tag="w_l1_f")
            w_l2_f = cpool.tile([128, 1024], F32, tag="w_l2_f")
            w_l0_f = cpool.tile([128, 2, 1024], F32, tag="w_l0_f")
            w_head_f = cpool.tile([128, 8, 1024], F32, tag="w_head_f")
            hwT_r = p_hwT.rearrange("(k p) d -> p k d", p=128)
            l0_r = p_l0.rearrange("(k p) d -> p k d", p=128)

            nc.scalar.dma_start(out=w_l1_f[0:64, :], in_=p_l1[:])
            nc.scalar.dma_start(out=w_l1_f[64:128, :], in_=p_l1[:])
            for j in range(8):
                nc.sync.dma_start(out=w_l2_f[16 * j:16 * j + 16, :], in_=p_l2[:])
            nc.scalar.dma_start(out=w_l0_f[:], in_=l0_r[:])
            nc.scalar.dma_start(out=w_head_f[:], in_=hwT_r[:])
            nc.scalar.copy(out=w_l1[:], in_=w_l1_f[:])
            nc.scalar.copy(out=w_l2[:], in_=w_l2_f[:])
            nc.gpsimd.tensor_copy(out=w_l0[:], in_=w_l0_f[:])
            nc.gpsimd.tensor_copy(out=w_head[:], in_=w_head_f[:])

            # ---- PE clock warmup: fp32 matmuls during the gather window --
            # (depends only on ident; shares the ptb ring slot)
            warm = ptb_pool.tile([128, 512], F32, tag="ptb")
            for _ in range(10):
                nc.tensor.matmul(out=warm[:, :128], lhsT=ident[:],
                                 rhs=ident[:], start=True, stop=True)

            def rhs_for(g, sub):
                cl = g["cluster"]
                if cl == 0:
                    return lambda k, sl: w_head[:, k, sl]
                if cl == 1:
                    return lambda k, sl: w_l0[:, k, sl]
                if cl == 2:
                    return lambda k, sl: w_l1[64 * sub:64 * sub + 64, sl]
                return lambda k, sl: w_l2[64 * sub:64 * sub + 64, sl]

            # ---- transpose units ----------------------------------------
            # h=64 groups: unit u = tile pair (2u, 2u+1) -> one [128,128]
            #   transpose (tile A rows 0:64, tile B rows 64:128).
            # h>=128 groups: unit = (tile c, k-slice) -> [128,128].
            units = []   # (gi, u_or_c, k, ncols)
            tiles = []   # (gi, c)
            for gi, g in enumerate(groups):
                h_eff = xg_tiles[gi][1]
                K = -(-h_eff // 128)
                g["K"] = K
                g["paired"] = h_eff == 64
                for c in range(g["C"]):
                    tiles.append((gi, c))
                if g["paired"]:
                    for u in range(-(-g["C"] // 2)):
                        nt = min(2, g["C"] - 2 * u)
                        units.append((gi, u, 0, nt * 64))
                else:
                    for c in range(g["C"]):
                        for k in range(K):
                            units.append((gi, c, k, 128))

            batches = []
            cur = []
            for u in units:
                if cur and (len(cur) == 4 or cur[0][0] != u[0]):
                    batches.append(cur)
                    cur = []
                cur.append(u)
            if cur:
                batches.append(cur)

            upos = {}
            bt0 = {}  # batch -> first consumer tile index (global)
            gt0 = {}
            t_acc = 0
            for gi, g in enumerate(groups):
                gt0[gi] = t_acc
                t_acc += g["C"]
            for bi, b in enumerate(batches):
                for ui, u in enumerate(b):
                    upos[(u[0], u[1], u[2])] = (bi, ui * 128)
                gi, uc = b[0][0], b[0][1]
                first_c = 2 * uc if groups[gi]["paired"] else uc
                bt0[bi] = gt0[gi] + first_c

            xt_of_batch = {}

            # logical-time floors steer the sim-driven Tile scheduler: they
            # stop it hoisting later groups' transposes (and their gather
            # waits) ahead of earlier groups' matmuls, which on HW stalled
            # the whole PE stream on the slowest gather.
            TICK = 0.002  # ms of model time per output tile

            def emit_batch(bi):
                b = batches[bi]
                gi = b[0][0]
                xg, h_eff = xg_tiles[gi]
                g = groups[gi]
                kk = 128 if g["paired"] else min(128, h_eff)
                used = len(b) * 128
                with tc.tile_wait_until(bt0[bi] * TICK):
                    ptb = ptb_pool.tile([128, 512], F32, tag="ptb")
                    for ui, (gi_, uc, k, ncols) in enumerate(b):
                        if g["paired"]:
                            nt = ncols // 64
                            in_ = xg[:, 2 * uc:2 * uc + nt, :]
                        else:
                            cw = min(128, h_eff - 128 * k)
                            in_ = xg[:, uc, 128 * k:128 * k + cw]
                            ncols = cw
                        nc.tensor.transpose(
                            out=ptb[:ncols, 128 * ui:128 * ui + 128],
                            in_=in_,
                            identity=ident[:],
                        )
                    xt = xtpool.tile([128, 512], BF16, tag="xt")
                    if g["quad"]:
                        u0 = b[0][1]
                        nc.vector.tensor_tensor(
                            out=xt[:kk, :used], in0=ptb[:kk, :used],
                            in1=mask_sb[:, 128 * u0:128 * u0 + used],
                            op=mybir.AluOpType.mult,
                        )
                    else:
                        nc.vector.tensor_copy(out=xt[:kk, :used],
                                              in_=ptb[:kk, :used])
                    xt_of_batch[bi] = xt

            emitted = 0

            def ensure_batch(bi):
                nonlocal emitted
                while emitted <= bi + 1 and emitted < len(batches):
                    emit_batch(emitted)
                    emitted += 1

            copy_alt = 1  # start on ACT: DVE also does the batch casts
            stage = None
            chunk0 = 0
            for ti, (gi, c) in enumerate(tiles):
                g = groups[gi]
                xg, h_eff = xg_tiles[gi]
                K = g["K"]

                if g["paired"]:
                    ensure_batch(upos[(gi, c // 2, 0)][0])
                else:
                    ensure_batch(upos[(gi, c, K - 1)][0])
                with tc.tile_wait_until((ti + 2) * TICK):
                    po = po_pool.tile([128, 1024], F32, tag="po")
                    if g["paired"]:
                        u, sub = c // 2, c % 2
                        bi, col = upos[(gi, u, 0)]
                        xt = xt_of_batch[bi]
                        rhs = rhs_for(g, sub)
                        lo = 64 * sub
                        for n in range(1024 // MM_N):
                            sl = slice(MM_N * n, MM_N * (n + 1))
                            nc.tensor.matmul(
                                out=po[:, sl],
                                lhsT=xt[lo:lo + 64, col:col + 128],
                                rhs=rhs(0, sl), start=True, stop=True,
                            )
                    else:
                        rhs = rhs_for(g, 0)
                        for k in range(K):
                            bi, col = upos[(gi, c, k)]
                            xt = xt_of_batch[bi]
                            for n in range(1024 // MM_N):
                                sl = slice(MM_N * n, MM_N * (n + 1))
                                nc.tensor.matmul(
                                    out=po[:, sl], lhsT=xt[:128, col:col + 128],
                                    rhs=rhs(k, sl),
                                    start=(k == 0), stop=(k == K - 1),
                                )

                    slot = ti - chunk0
                    if slot == 0:
                        stage = stpool.tile([128, ST, 1024], BF16, tag="stage")
                    if copy_alt % 2 == 0:
                        nc.vector.tensor_copy(out=stage[:, slot, :], in_=po[:])
                    else:
                        nc.scalar.copy(out=stage[:, slot, :], in_=po[:])
                    copy_alt += 1

                    if slot == ST - 1 or ti == len(tiles) - 1:
                        nw = slot + 1
                        nc.sync.dma_start(
                            out=p_out[:, chunk0:chunk0 + nw, :],
                            in_=stage[:, :nw, :],
                        )
                        chunk0 = ti + 1

    nc.compile()
    return nc


_GRAPH_CACHE = {}


def _prepare(tokens_flat):
    groups, per_core, T_total = _plan(tokens_flat)

    key = tuple(g["cap"] for g in groups)
    if key not in _GRAPH_CACHE:
        _GRAPH_CACHE[key] = _build_graph(groups, T_total)
    nc = _GRAPH_CACHE[key]

    C2 = next(g["C"] for g in groups if g["quad"])
    U2 = -(-C2 // 2)
    gidx_np, mask_np = [], []
    for i in range(N_CORES):
        gcols = []
        mask = np.zeros((128, U2 * 128), np.float32)
        for gi, g in enumerate(groups):
            sel, loc = per_core[i][gi]
            if g["quad"]:
                gvals = loc // 4
                sub = loc % 4
                for s_i, ssub in enumerate(sub):
                    p, c = s_i % 128, s_i // 128
                    row = 64 * (c % 2) + 16 * ssub
                    mask[row:row + 16, 128 * (c // 2) + p] = 1.0
            else:
                gvals = loc
            gcols.append(_wrap16(gvals, g["cap"], PAD_IDX))
        gidx_np.append(np.concatenate(gcols, axis=1))
        mask_np.append(mask)
    return nc, groups, per_core, T_total, gidx_np, mask_np


def run(inputs, trace=False):
    tokens = np.asarray(inputs["tokens"])
    tokens_flat = tokens.reshape(-1).astype(np.int64)
    nc, groups, per_core, T_total, gidx_np, mask_np = _prepare(tokens_flat)

    head_wT = np.ascontiguousarray(np.asarray(inputs["head_w"]).T)
    shared = {
        "head_emb": np.asarray(inputs["head_emb"], np.float32),
        "tail_emb0": np.asarray(inputs["tail_emb0"], np.float32),
        "tail_emb1": np.asarray(inputs["tail_emb1"], np.float32),
        "tail_emb2": np.asarray(inputs["tail_emb2"], np.float32),
        "head_wT": head_wT.astype(np.float32),
        "tail_lin0": np.asarray(inputs["tail_lin0"], np.float32),
        "tail_lin1": np.asarray(inputs["tail_lin1"], np.float32),
        "tail_lin2": np.asarray(inputs["tail_lin2"], np.float32),
        "ident": np.eye(128, dtype=np.float32),
    }
    in_maps = []
    for i in range(N_CORES):
        m = dict(shared)
        m["gidx"] = gidx_np[i]
        m["maskT2"] = mask_np[i]
        in_maps.append(m)

    res = None
    for attempt in range(3):
        try:
            res = run_bass_kernel_spmd(nc, in_maps, core_ids=list(range(N_CORES)),
                                       trace=trace)
            break
        except Exception:
            if attempt == 2:
                raise
            import time
            time.sleep(2)

    out_flat = np.empty((N_CORES * N_TOK, D), np.float32)
    for i in range(N_CORES):
        r = res.results[i]["out"]  # [128, T_total, 1024] bf16
        for gi, g in enumerate(groups):
            sel, _ = per_core[i][gi]
            n = len(sel)
            if n:
                s = np.arange(n)
                out_flat[sel] = r[s % 128, g["tile0"] + s // 128].astype(np.float32)
    return out_flat.reshape(tokens.shape[0], tokens.shape[1], D), res


def kernel(**inputs):
    out, _ = run(inputs, trace=False)
    return out
